# revision 11
# baseline (speedup 1.0000x reference)
"""Trainium2 Bass kernel for nn_MultiHeadAttention (B=4, S=2048, DIM=768,
EMBED=512, HEADS=8, HEAD_DIM=64), distributed over 8 NeuronCores.

Sharding: core (b, g), b in 0..3 (batch), g in 0..1 (head-group of 4 heads).
Host sums the two group partials per batch and adds the effective output
bias (bo + bv @ wo; bk is dropped — a per-row-constant logit shift is
softmax-invariant).

v3 (bf16 everywhere; fp8 was 4x over the error budget):
  - PV chunks are emitted through a global lagged work queue so the
    in-order PE queue never head-blocks on ACT (v1 lost ~1.1us/iter).
  - The PV accumulator (single PSUM buffer) is released by 4 plain DVE
    copies (U and R halves) right after a block's last PV; the Newton
    reciprocal (gpsimd), partition-shift DMA (sync) and final multiply
    (DVE) run off the critical path.
  - exp on ACT, FD=1024 per key-chunk, es in an 8-slot bf16 ring.
  - Preamble: DMA rings ordered for earliest first exp (wk,wq then xk,
    xq query-block-0 slice, then xv, then the rest of xq); K proj
    m-major; V projections and block-0 PV drain in once xv lands.
  - Tail: last block's out-projection immediately after its normalize.
"""

import numpy as np
import ml_dtypes

import concourse.bass as bass
import concourse.tile as tile
from concourse import mybir
from concourse.bass_utils import run_bass_kernel_spmd

BF16 = mybir.dt.bfloat16
F32 = mybir.dt.float32
NPBF16 = ml_dtypes.bfloat16

B, S, DIM, EMBED, HEADS, HEAD_DIM = 4, 2048, 768, 512, 8, 64
P = 128
KD = DIM // P
GROUPS = 2
GE = EMBED // GROUPS   # 256
GH = HEADS // GROUPS   # 4
MQ = GE // P           # 2
SC = S // P            # 16
NB = 512
NQ = S // NB           # 4
SCALE = 0.125
NCORES = B * GROUPS
ESR = 8                # es ring slots
X0 = 1.0 / 2146.0      # Newton seed for 1/rowsum


def _split_multi_waits(nc):
    """This image's walrus accepts at most ONE sem-wait per instruction.
    Hoist all but the last wait onto same-engine NoOps; replace the tail
    gpsimd RANGE_CLEAR (rejected encoding) with a NoOp."""
    ctr = 0
    for f in nc.m.functions:
        for blk in f.blocks:
            il = blk.instructions
            out = []
            for inst in il:
                if type(inst).__name__ == "InstISA":
                    nop = mybir.InstNoOp(
                        name=f"{inst.name}-isanop", ins=[], outs=[]
                    )
                    nop.engine = inst.engine
                    nop.sync_info = inst.sync_info
                    out.append(nop)
                    continue
                si = inst.sync_info
                if si is not None and si.on_wait and len(si.on_wait) > 1:
                    waits = list(si.on_wait)
                    for w in waits[:-1]:
                        ctr += 1
                        nop = mybir.InstNoOp(
                            name=f"I-waitsplit-{ctr}", ins=[], outs=[]
                        )
                        nop.engine = inst.engine
                        nop.sync_info = mybir.SyncInfo(on_wait=[w], on_update=[])
                        out.append(nop)
                    si.on_wait = [waits[-1]]
                out.append(inst)
            il[:] = out
    return ctr


def build_nc(split_waits=True):
    nc = bass.Bass("TRN2", target_bir_lowering=False, debug=False)

    xqT = nc.dram_tensor("xqT", [DIM, S], BF16, kind="ExternalInput").ap()
    xkT = nc.dram_tensor("xkT", [DIM, S], BF16, kind="ExternalInput").ap()
    xvT = nc.dram_tensor("xvT", [DIM, S], BF16, kind="ExternalInput").ap()
    wq = nc.dram_tensor("wq", [DIM, GE], BF16, kind="ExternalInput").ap()
    wk = nc.dram_tensor("wk", [DIM, GE], BF16, kind="ExternalInput").ap()
    wv = nc.dram_tensor("wv", [DIM, GE], BF16, kind="ExternalInput").ap()
    wo = nc.dram_tensor("wo", [GE, DIM], BF16, kind="ExternalInput").ap()
    bq = nc.dram_tensor("bq", [GE], F32, kind="ExternalInput").ap()
    out = nc.dram_tensor("out", [S, DIM], BF16, kind="ExternalOutput").ap()

    add = mybir.AluOpType.add
    mult = mybir.AluOpType.mult
    Exp = mybir.ActivationFunctionType.Exp

    with tile.TileContext(nc) as tc:
        with (
            tc.tile_pool(name="const", bufs=1) as const,
            tc.tile_pool(name="psS", bufs=2, space="PSUM") as psS,
            tc.tile_pool(name="psPU", bufs=1, space="PSUM") as psPU,
            tc.tile_pool(name="psM", bufs=2, space="PSUM") as psM,
            tc.tile_pool(name="nrm", bufs=2) as nrm,
            tc.tile_pool(name="yout", bufs=2) as yout,
        ):
            wq_sb = const.tile([P, KD, GE], BF16, tag="wq")
            wk_sb = const.tile([P, KD, GE], BF16, tag="wk")
            wv_sb = const.tile([P, KD, GE], BF16, tag="wv")
            wo_sb = const.tile([P, MQ, DIM], BF16, tag="wo")
            bq_sb = const.tile([P, MQ], F32, tag="bq")
            xq_sb = const.tile([P, KD, S], BF16, tag="xq")
            xk_sb = const.tile([P, KD, S], BF16, tag="xk")
            xv_sb = const.tile([P, KD, S], BF16, tag="xv")
            qt_sb = const.tile([P, MQ, S], BF16, tag="qt")
            kt_sb = const.tile([P, MQ, S], BF16, tag="kt")
            ot_sb = const.tile([P, MQ, S], BF16, tag="ot")
            es_sb = const.tile([P, ESR, 2, NB], BF16, tag="es")
            v_sb = const.tile([P, SC, GH, P], BF16, tag="v")
            # only the ones-halves need the memset; V halves get overwritten
            nc.vector.memset(v_sb[:, :, 0::2, HEAD_DIM:P], 1.0)
            nc.vector.memset(v_sb[:, :, 1::2, 0:HEAD_DIM], 1.0)

            # --- input DMAs.  sync: weights; gpsimd: xk, xq(q0 cols), xv,
            # xq(rest) — queue order is service order, so the first-exp
            # critical bytes (wk,wq,xk,xq[:,:512]) lead the bus.
            nc.sync.dma_start(wk_sb[:], wk.rearrange("(k p) e -> p k e", p=P))
            nc.sync.dma_start(wq_sb[:], wq.rearrange("(k p) e -> p k e", p=P))
            nc.sync.dma_start(bq_sb[:], bq.rearrange("(m p) -> p m", p=P))
            nc.sync.dma_start(wv_sb[:], wv.rearrange("(k p) e -> p k e", p=P))
            nc.sync.dma_start(wo_sb[:], wo.rearrange("(m p) d -> p m d", p=P))
            xkr = xkT.rearrange("(k p) s -> p k s", p=P)
            xqr = xqT.rearrange("(k p) s -> p k s", p=P)
            xvr = xvT.rearrange("(k p) s -> p k s", p=P)
            nc.gpsimd.dma_start(xk_sb[:], xkr)
            nc.gpsimd.dma_start(xq_sb[:, :, 0:NB], xqr[:, :, 0:NB])
            nc.gpsimd.dma_start(xv_sb[:], xvr)
            nc.gpsimd.dma_start(xq_sb[:, :, NB:S], xqr[:, :, NB:S])

            # --- building blocks ---
            qp_hold = {}

            def qk_proj_block(x_sb, w_sb, dst, m, n, with_bias, ks=None):
                if ks is None:
                    ks = range(KD)
                key = (dst.name, m, n)
                if ks[0] == 0:
                    qp_hold[key] = psM.tile([P, NB], F32, tag="m",
                                            name=f"pj{dst.name}{m}_{n}")
                ps = qp_hold[key]
                for k in ks:
                    nc.tensor.matmul(
                        ps[:],
                        lhsT=w_sb[:, k, m * P:(m + 1) * P],
                        rhs=x_sb[:, k, n * NB:(n + 1) * NB],
                        start=(k == 0), stop=(k == KD - 1),
                    )
                if ks[-1] != KD - 1:
                    return
                del qp_hold[key]
                if with_bias:
                    nc.vector.tensor_scalar(
                        out=dst[:, m, n * NB:(n + 1) * NB], in0=ps[:],
                        scalar1=bq_sb[:, m:m + 1], scalar2=None, op0=add,
                    )
                else:
                    nc.vector.tensor_copy(dst[:, m, n * NB:(n + 1) * NB], ps[:])

            def v_proj_chunk(s):
                ps = psM.tile([P, NB], F32, tag="m", name=f"pv{s}")
                for k in range(KD):
                    nc.tensor.matmul(
                        ps[:, 0:GE],
                        lhsT=xv_sb[:, k, s * P:(s + 1) * P],
                        rhs=wv_sb[:, k, :],
                        start=(k == 0), stop=(k == KD - 1),
                    )
                ps_h = ps[:, 0:GE].rearrange("p (h d) -> p h d", d=HEAD_DIM)
                nc.vector.tensor_copy(v_sb[:, s, 0::2, 0:HEAD_DIM],
                                      ps_h[:, 0::2, :])
                nc.vector.tensor_copy(v_sb[:, s, 1::2, HEAD_DIM:P],
                                      ps_h[:, 1::2, :])

            def out_proj_half(s, half):
                lo, hi = (0, NB) if half == 0 else (NB, DIM)
                py = psM.tile([P, NB], F32, tag="m", name=f"py{s}_{half}")
                for k in range(MQ):
                    nc.tensor.matmul(
                        py[:, 0:hi - lo],
                        lhsT=ot_sb[:, k, s * P:(s + 1) * P],
                        rhs=wo_sb[:, k, lo:hi],
                        start=(k == 0), stop=(k == MQ - 1),
                    )
                if half == 0:
                    out_proj_half.y[s] = yout.tile([P, DIM], BF16, tag="y",
                                                   name=f"y{s}")
                y_sb = out_proj_half.y[s]
                nc.vector.tensor_copy(y_sb[:, lo:hi], py[:, 0:hi - lo])
                if half == 1:
                    nc.sync.dma_start(out[s * P:(s + 1) * P, :], y_sb[:])
            out_proj_half.y = {}
            out_proj_half.done = {q: 0 for q in range(NQ)}

            def normalize(pu, hp, q):
                """Free pu via 4 DVE copies, then recip + multiply off the
                critical path (gpsimd Newton, sync shift-DMA, one DVE op).
                For the last q the chain latency is the kernel tail, so
                Newton runs on DVE and the shifts use two rings."""
                eng = nc.vector if q == NQ - 1 else nc.gpsimd
                rt = nrm.tile([P, NB], F32, tag="rt", name=f"rt{hp}_{q}")
                uc = nrm.tile([P, NB], F32, tag="uc", name=f"uc{hp}_{q}")
                x1 = nrm.tile([P, NB], F32, tag="x1", name=f"x1{hp}_{q}")
                tm = nrm.tile([P, NB], F32, tag="tm", name=f"tm{hp}_{q}")
                xr = nrm.tile([P, NB], F32, tag="xr", name=f"xr{hp}_{q}")
                nc.vector.tensor_copy(uc[0:64, :], pu[0:64, 0, :])
                nc.vector.tensor_copy(uc[64:128, :], pu[64:128, 1, :])
                nc.vector.tensor_copy(rt[64:128, :], pu[64:128, 0, :])
                nc.vector.tensor_copy(rt[0:64, :], pu[0:64, 1, :])
                eng.tensor_scalar(   # x1 = 2x0 - x0^2 r
                    out=x1[:], in0=rt[:], scalar1=-X0 * X0,
                    scalar2=2.0 * X0, op0=mult, op1=add,
                )
                eng.tensor_tensor(   # e = r * x1
                    out=tm[:], in0=rt[:], in1=x1[:], op=mult,
                )
                eng.tensor_scalar(   # u = 2 - e
                    out=tm[:], in0=tm[:], scalar1=-1.0, scalar2=2.0,
                    op0=mult, op1=add,
                )
                eng.tensor_tensor(   # x2 = x1 * u
                    out=x1[:], in0=x1[:], in1=tm[:], op=mult,
                )
                if q == NQ - 1 and hp == 1:
                    nc.scalar.dma_start(xr[0:64, :], x1[64:128, :])
                else:
                    nc.sync.dma_start(xr[0:64, :], x1[64:128, :])
                nc.sync.dma_start(xr[64:128, :], x1[0:64, :])
                qs = slice(q * NB, (q + 1) * NB)
                nc.vector.tensor_tensor(out=ot_sb[:, hp, qs], in0=uc[:],
                                        in1=xr[:], op=mult)

            # --- PE pstate warmup on junk data while input DMAs run ---
            wup = psM.tile([P, NB], F32, tag="m", name="warmup")
            for _ in range(24):
                nc.tensor.matmul(wup[:], lhsT=v_sb[:, 0, 0, :],
                                 rhs=v_sb[:, 0:4, 0, :],
                                 start=True, stop=True)

            # --- preamble projections: only what the first scores need ---
            qk_proj_block(xk_sb, wk_sb, kt_sb, 0, 0, False)
            qk_proj_block(xq_sb, wq_sb, qt_sb, 0, 0, True)

            # --- attention with global lagged PV queue ---
            state = {"pv": 0, "v": 0, "pu": {}, "oq": []}
            PVLAG, VLAG = 4, 6

            def emit_pv_chunk(i):
                pblk, c = divmod(i, SC)
                pq, php = divmod(pblk, MQ)
                if c == 0:
                    state["pu"][pblk] = psPU.tile([P, 2, NB], F32, tag="pu",
                                                  name=f"pu{pblk}")
                pu = state["pu"][pblk]
                for j in range(2):
                    nc.tensor.matmul(
                        pu[:, j, :],
                        lhsT=v_sb[:, c, 2 * php + j, :],
                        rhs=es_sb[:, c % ESR, j, :],
                        start=(c == 0), stop=(c == SC - 1),
                    )
                if c == SC - 1:
                    normalize(pu, php, pq)
                    del state["pu"][pblk]
                    if php == 1:
                        state["oq"].append(pq)

            def drain(gtime, max_pv):
                # V projections: one per tick once xv has landed
                if state["v"] < SC and gtime >= VLAG:
                    v_proj_chunk(state["v"])
                    state["v"] += 1
                n = 0
                while state["pv"] < NCORES * SC and n < max_pv:
                    i = state["pv"]
                    pblk, c = divmod(i, SC)
                    if SC * pblk + c > gtime - PVLAG:
                        break
                    if c >= state["v"]:
                        break
                    emit_pv_chunk(i)
                    state["pv"] += 1
                    n += 1

            for q in range(NQ):
                for hp in range(MQ):
                    blk = q * MQ + hp
                    for m in range(SC):
                        gtime = SC * blk + m
                        ss = psS.tile([P, 2, NB], F32, tag="s")
                        for j in range(2):
                            lo, hi = j * HEAD_DIM, (j + 1) * HEAD_DIM
                            nc.tensor.matmul(
                                ss[:, j, :],
                                lhsT=kt_sb[lo:hi, hp, m * P:(m + 1) * P],
                                rhs=qt_sb[lo:hi, hp, q * NB:(q + 1) * NB],
                                start=True, stop=True,
                            )
                        nc.scalar.activation(es_sb[:, m % ESR, :, :], ss[:],
                                             Exp, scale=SCALE)
                        if blk == 0 and 1 <= m <= 3:
                            qk_proj_block(xk_sb, wk_sb, kt_sb, 0, m, False)
                        if blk == 0 and 4 <= m <= 7:
                            qk_proj_block(xk_sb, wk_sb, kt_sb, 1, m - 4, False)
                        if blk == 0 and m == 8:
                            qk_proj_block(xq_sb, wq_sb, qt_sb, 1, 0, True)
                        if hp == 1 and q + 1 < NQ and 5 <= m <= 7:
                            qk_proj_block(xq_sb, wq_sb, qt_sb, 0, q + 1, True,
                                          ks=range(2 * (m - 5), 2 * (m - 4)))
                        if hp == 1 and q + 1 < NQ and 11 <= m <= 13:
                            qk_proj_block(xq_sb, wq_sb, qt_sb, 1, q + 1, True,
                                          ks=range(2 * (m - 11), 2 * (m - 10)))
                        drain(gtime, 2)
                        if m % 2 == 1 and m >= 5 and state["oq"]:
                            qd = state["oq"][0]
                            if out_proj_half.done[qd] < 8:
                                nn = out_proj_half.done[qd]
                                out_proj_half(qd * 4 + nn // 2, nn % 2)
                                out_proj_half.done[qd] = nn + 1
                            else:
                                state["oq"].pop(0)

            # --- tail: remaining PV chunks, normalizes, out-projections ---
            gtime = NCORES * SC
            while state["pv"] < NCORES * SC:
                drain(gtime, 2)
                gtime += 1
            for q in range(NQ):
                while out_proj_half.done[q] < 8:
                    nn = out_proj_half.done[q]
                    out_proj_half(q * 4 + nn // 2, nn % 2)
                    out_proj_half.done[q] = nn + 1

    if split_waits:
        _split_multi_waits(nc)
    return nc


_NC = None


def _get_nc():
    global _NC
    if _NC is None:
        _NC = build_nc()
    return _NC


def _bf(a):
    return np.ascontiguousarray(np.asarray(a, dtype=np.float32)).astype(NPBF16)


def make_in_maps(query, key, value, wq, bq, wk, bk, wv, bv, wo, bo):
    query = np.asarray(query, np.float32)
    key = np.asarray(key, np.float32)
    value = np.asarray(value, np.float32)
    wqf = np.asarray(wq, np.float32)
    wkf = np.asarray(wk, np.float32)
    wvf = np.asarray(wv, np.float32)
    wof = np.asarray(wo, np.float32)
    in_maps = []
    for b in range(B):
        xqT = _bf(query[b].T)
        xkT = _bf(key[b].T)
        xvT = _bf(value[b].T)
        for g in range(GROUPS):
            sl = slice(g * GE, (g + 1) * GE)
            in_maps.append({
                "xqT": xqT,
                "xkT": xkT,
                "xvT": xvT,
                "wq": _bf(wqf[:, sl]),
                "wk": _bf(wkf[:, sl]),
                "wv": _bf(wvf[:, sl]),
                "wo": _bf(wof[sl, :]),
                "bq": np.ascontiguousarray(np.asarray(bq, np.float32)[sl]),
            })
    return in_maps


def kernel(query, key, value, wq, bq, wk, bk, wv, bv, wo, bo, **kw):
    nc = _get_nc()
    in_maps = make_in_maps(query, key, value, wq, bq, wk, bk, wv, bv, wo, bo)
    res = run_bass_kernel_spmd(nc, in_maps, list(range(NCORES))).results
    # bk is softmax-invariant; bv rides through softmax into a constant
    bo_eff = (np.asarray(bo, np.float32)
              + np.asarray(bv, np.float32) @ np.asarray(wo, np.float32))
    outp = np.empty((B, S, DIM), np.float32)
    for b in range(B):
        outp[b] = (res[b * GROUPS]["out"].astype(np.float32)
                   + res[b * GROUPS + 1]["out"].astype(np.float32) + bo_eff)
    return outp


# revision 12
# speedup vs baseline: 1.1478x; 1.1478x over previous
"""Trainium2 Bass kernel for nn_MultiHeadAttention (B=4, S=2048, DIM=768,
EMBED=512, HEADS=8, HEAD_DIM=64), distributed over 8 NeuronCores.

Sharding: core (b, g), b in 0..3 (batch), g in 0..1 (head-group of 4 heads).
Host sums the two group partials per batch and adds the effective output
bias (bo + bv @ wo; bk is dropped — a per-row-constant logit shift is
softmax-invariant).

v3 (bf16 everywhere; fp8 was 4x over the error budget):
  - PV chunks are emitted through a global lagged work queue so the
    in-order PE queue never head-blocks on ACT (v1 lost ~1.1us/iter).
  - The PV accumulator (single PSUM buffer) is released by 4 plain DVE
    copies (U and R halves) right after a block's last PV; the Newton
    reciprocal (gpsimd), partition-shift DMA (sync) and final multiply
    (DVE) run off the critical path.
  - exp on ACT, FD=1024 per key-chunk, es in an 8-slot bf16 ring.
  - Preamble: DMA rings ordered for earliest first exp (wk,wq then xk,
    xq query-block-0 slice, then xv, then the rest of xq); K proj
    m-major; V projections and block-0 PV drain in once xv lands.
  - Tail: last block's out-projection immediately after its normalize.
"""

import numpy as np
import ml_dtypes

import concourse.bass as bass
import concourse.tile as tile
from concourse import mybir
from concourse.bass_utils import run_bass_kernel_spmd

BF16 = mybir.dt.bfloat16
F32 = mybir.dt.float32
NPBF16 = ml_dtypes.bfloat16

B, S, DIM, EMBED, HEADS, HEAD_DIM = 4, 2048, 768, 512, 8, 64
P = 128
KD = DIM // P
GROUPS = 2
GE = EMBED // GROUPS   # 256
GH = HEADS // GROUPS   # 4
MQ = GE // P           # 2
SC = S // P            # 16
NB = 512
NQ = S // NB           # 4
SCALE = 0.125
NCORES = B * GROUPS
ESR = 8                # es ring slots
X0 = 1.0 / 2146.0      # Newton seed for 1/rowsum


def _split_multi_waits(nc):
    """This image's walrus accepts at most ONE sem-wait per instruction.
    Hoist all but the last wait onto same-engine NoOps; replace the tail
    gpsimd RANGE_CLEAR (rejected encoding) with a NoOp."""
    ctr = 0
    for f in nc.m.functions:
        for blk in f.blocks:
            il = blk.instructions
            out = []
            for inst in il:
                if type(inst).__name__ == "InstISA":
                    nop = mybir.InstNoOp(
                        name=f"{inst.name}-isanop", ins=[], outs=[]
                    )
                    nop.engine = inst.engine
                    nop.sync_info = inst.sync_info
                    out.append(nop)
                    continue
                si = inst.sync_info
                if si is not None and si.on_wait and len(si.on_wait) > 1:
                    waits = list(si.on_wait)
                    for w in waits[:-1]:
                        ctr += 1
                        nop = mybir.InstNoOp(
                            name=f"I-waitsplit-{ctr}", ins=[], outs=[]
                        )
                        nop.engine = inst.engine
                        nop.sync_info = mybir.SyncInfo(on_wait=[w], on_update=[])
                        out.append(nop)
                    si.on_wait = [waits[-1]]
                out.append(inst)
            il[:] = out
    return ctr


def build_nc(split_waits=True):
    nc = bass.Bass("TRN2", target_bir_lowering=False, debug=False)

    xqT = nc.dram_tensor("xqT", [DIM, S], BF16, kind="ExternalInput").ap()
    xkT = nc.dram_tensor("xkT", [DIM, S], BF16, kind="ExternalInput").ap()
    xvT = nc.dram_tensor("xvT", [DIM, S], BF16, kind="ExternalInput").ap()
    wq = nc.dram_tensor("wq", [DIM, GE], BF16, kind="ExternalInput").ap()
    wk = nc.dram_tensor("wk", [DIM, GE], BF16, kind="ExternalInput").ap()
    wv = nc.dram_tensor("wv", [DIM, GE], BF16, kind="ExternalInput").ap()
    wo = nc.dram_tensor("wo", [GE, DIM], BF16, kind="ExternalInput").ap()
    bq = nc.dram_tensor("bq", [GE], F32, kind="ExternalInput").ap()
    out = nc.dram_tensor("out", [S, DIM], BF16, kind="ExternalOutput").ap()

    add = mybir.AluOpType.add
    mult = mybir.AluOpType.mult
    Exp = mybir.ActivationFunctionType.Exp

    with tile.TileContext(nc) as tc:
        with (
            tc.tile_pool(name="const", bufs=1) as const,
            tc.tile_pool(name="psS", bufs=2, space="PSUM") as psS,
            tc.tile_pool(name="psPU", bufs=1, space="PSUM") as psPU,
            tc.tile_pool(name="psM", bufs=2, space="PSUM") as psM,
            tc.tile_pool(name="nrm", bufs=2) as nrm,
            tc.tile_pool(name="yout", bufs=2) as yout,
        ):
            wq_sb = const.tile([P, KD, GE], BF16, tag="wq")
            wk_sb = const.tile([P, KD, GE], BF16, tag="wk")
            wv_sb = const.tile([P, KD, GE], BF16, tag="wv")
            wo_sb = const.tile([P, MQ, DIM], BF16, tag="wo")
            bq_sb = const.tile([P, MQ], F32, tag="bq")
            xq_sb = const.tile([P, KD, S], BF16, tag="xq")
            xk_sb = const.tile([P, KD, S], BF16, tag="xk")
            xv_sb = const.tile([P, KD, S], BF16, tag="xv")
            qt_sb = const.tile([P, MQ, S], BF16, tag="qt")
            kt_sb = const.tile([P, MQ, S], BF16, tag="kt")
            ot_sb = const.tile([P, MQ, S], BF16, tag="ot")
            es_sb = const.tile([P, ESR, 2, NB], BF16, tag="es")
            v_sb = const.tile([P, SC, GH, P], BF16, tag="v")
            # only the ones-halves need the memset; V halves get overwritten
            nc.vector.memset(v_sb[:, :, 0::2, HEAD_DIM:P], 1.0)
            nc.vector.memset(v_sb[:, :, 1::2, 0:HEAD_DIM], 1.0)

            # --- input DMAs.  sync: weights; gpsimd: xk, xq(q0 cols), xv,
            # xq(rest) — queue order is service order, so the first-exp
            # critical bytes (wk,wq,xk,xq[:,:512]) lead the bus.
            nc.sync.dma_start(wk_sb[:], wk.rearrange("(k p) e -> p k e", p=P))
            nc.sync.dma_start(wq_sb[:], wq.rearrange("(k p) e -> p k e", p=P))
            nc.sync.dma_start(bq_sb[:], bq.rearrange("(m p) -> p m", p=P))
            nc.sync.dma_start(wv_sb[:], wv.rearrange("(k p) e -> p k e", p=P))
            nc.sync.dma_start(wo_sb[:], wo.rearrange("(m p) d -> p m d", p=P))
            xkr = xkT.rearrange("(k p) s -> p k s", p=P)
            xqr = xqT.rearrange("(k p) s -> p k s", p=P)
            xvr = xvT.rearrange("(k p) s -> p k s", p=P)
            nc.gpsimd.dma_start(xk_sb[:], xkr)
            nc.gpsimd.dma_start(xq_sb[:, :, 0:NB], xqr[:, :, 0:NB])
            nc.gpsimd.dma_start(xv_sb[:], xvr)
            nc.gpsimd.dma_start(xq_sb[:, :, NB:S], xqr[:, :, NB:S])

            # --- building blocks ---
            qp_hold = {}

            def qk_proj_block(x_sb, w_sb, dst, m, n, with_bias, ks=None):
                if ks is None:
                    ks = range(KD)
                key = (dst.name, m, n)
                if ks[0] == 0:
                    qp_hold[key] = psM.tile([P, NB], F32, tag="m",
                                            name=f"pj{dst.name}{m}_{n}")
                ps = qp_hold[key]
                for k in ks:
                    nc.tensor.matmul(
                        ps[:],
                        lhsT=w_sb[:, k, m * P:(m + 1) * P],
                        rhs=x_sb[:, k, n * NB:(n + 1) * NB],
                        start=(k == 0), stop=(k == KD - 1),
                    )
                if ks[-1] != KD - 1:
                    return
                del qp_hold[key]
                if with_bias:
                    nc.vector.tensor_scalar(
                        out=dst[:, m, n * NB:(n + 1) * NB], in0=ps[:],
                        scalar1=bq_sb[:, m:m + 1], scalar2=None, op0=add,
                    )
                else:
                    nc.vector.tensor_copy(dst[:, m, n * NB:(n + 1) * NB], ps[:])

            def v_proj_chunk(s):
                ps = psM.tile([P, NB], F32, tag="m", name=f"pv{s}")
                for k in range(KD):
                    nc.tensor.matmul(
                        ps[:, 0:GE],
                        lhsT=xv_sb[:, k, s * P:(s + 1) * P],
                        rhs=wv_sb[:, k, :],
                        start=(k == 0), stop=(k == KD - 1),
                    )
                ps_h = ps[:, 0:GE].rearrange("p (h d) -> p h d", d=HEAD_DIM)
                nc.vector.tensor_copy(v_sb[:, s, 0::2, 0:HEAD_DIM],
                                      ps_h[:, 0::2, :])
                nc.vector.tensor_copy(v_sb[:, s, 1::2, HEAD_DIM:P],
                                      ps_h[:, 1::2, :])

            def out_proj_half(s, half):
                lo, hi = (0, NB) if half == 0 else (NB, DIM)
                py = psM.tile([P, NB], F32, tag="m", name=f"py{s}_{half}")
                for k in range(MQ):
                    nc.tensor.matmul(
                        py[:, 0:hi - lo],
                        lhsT=ot_sb[:, k, s * P:(s + 1) * P],
                        rhs=wo_sb[:, k, lo:hi],
                        start=(k == 0), stop=(k == MQ - 1),
                    )
                if half == 0:
                    out_proj_half.y[s] = yout.tile([P, DIM], BF16, tag="y",
                                                   name=f"y{s}")
                y_sb = out_proj_half.y[s]
                nc.vector.tensor_copy(y_sb[:, lo:hi], py[:, 0:hi - lo])
                if half == 1:
                    nc.sync.dma_start(out[s * P:(s + 1) * P, :], y_sb[:])
            out_proj_half.y = {}
            out_proj_half.done = {q: 0 for q in range(NQ)}

            def normalize(pu, hp, q):
                """Free pu via 4 DVE copies, then recip + multiply off the
                critical path (gpsimd Newton, sync shift-DMA, one DVE op).
                For the last q the chain latency is the kernel tail, so
                Newton runs on DVE and the shifts use two rings."""
                eng = nc.vector if q == NQ - 1 else nc.gpsimd
                rt = nrm.tile([P, NB], F32, tag="rt", name=f"rt{hp}_{q}")
                uc = nrm.tile([P, NB], F32, tag="uc", name=f"uc{hp}_{q}")
                x1 = nrm.tile([P, NB], F32, tag="x1", name=f"x1{hp}_{q}")
                tm = nrm.tile([P, NB], F32, tag="tm", name=f"tm{hp}_{q}")
                xr = nrm.tile([P, NB], F32, tag="xr", name=f"xr{hp}_{q}")
                nc.vector.tensor_copy(uc[0:64, :], pu[0:64, 0, :])
                nc.vector.tensor_copy(uc[64:128, :], pu[64:128, 1, :])
                nc.vector.tensor_copy(rt[64:128, :], pu[64:128, 0, :])
                nc.vector.tensor_copy(rt[0:64, :], pu[0:64, 1, :])
                eng.tensor_scalar(   # x1 = 2x0 - x0^2 r
                    out=x1[:], in0=rt[:], scalar1=-X0 * X0,
                    scalar2=2.0 * X0, op0=mult, op1=add,
                )
                eng.tensor_tensor(   # e = r * x1
                    out=tm[:], in0=rt[:], in1=x1[:], op=mult,
                )
                eng.tensor_scalar(   # u = 2 - e
                    out=tm[:], in0=tm[:], scalar1=-1.0, scalar2=2.0,
                    op0=mult, op1=add,
                )
                eng.tensor_tensor(   # x2 = x1 * u
                    out=x1[:], in0=x1[:], in1=tm[:], op=mult,
                )
                if q == NQ - 1 and hp == 1:
                    nc.scalar.dma_start(xr[0:64, :], x1[64:128, :])
                else:
                    nc.sync.dma_start(xr[0:64, :], x1[64:128, :])
                nc.sync.dma_start(xr[64:128, :], x1[0:64, :])
                qs = slice(q * NB, (q + 1) * NB)
                nc.vector.tensor_tensor(out=ot_sb[:, hp, qs], in0=uc[:],
                                        in1=xr[:], op=mult)

            # --- preamble projections: only what the first scores need ---
            qk_proj_block(xk_sb, wk_sb, kt_sb, 0, 0, False)
            qk_proj_block(xq_sb, wq_sb, qt_sb, 0, 0, True)

            # --- attention with global lagged PV queue ---
            state = {"pv": 0, "v": 0, "pu": {}, "oq": []}
            PVLAG, VLAG = 4, 6

            def emit_pv_chunk(i):
                pblk, c = divmod(i, SC)
                pq, php = divmod(pblk, MQ)
                if c == 0:
                    state["pu"][pblk] = psPU.tile([P, 2, NB], F32, tag="pu",
                                                  name=f"pu{pblk}")
                pu = state["pu"][pblk]
                for j in range(2):
                    nc.tensor.matmul(
                        pu[:, j, :],
                        lhsT=v_sb[:, c, 2 * php + j, :],
                        rhs=es_sb[:, c % ESR, j, :],
                        start=(c == 0), stop=(c == SC - 1),
                    )
                if c == SC - 1:
                    normalize(pu, php, pq)
                    del state["pu"][pblk]
                    if php == 1:
                        state["oq"].append(pq)

            def drain(gtime, max_pv):
                # V projections: one per tick once xv has landed
                if state["v"] < SC and gtime >= VLAG:
                    v_proj_chunk(state["v"])
                    state["v"] += 1
                n = 0
                while state["pv"] < NCORES * SC and n < max_pv:
                    i = state["pv"]
                    pblk, c = divmod(i, SC)
                    if SC * pblk + c > gtime - PVLAG:
                        break
                    if c >= state["v"]:
                        break
                    emit_pv_chunk(i)
                    state["pv"] += 1
                    n += 1

            for q in range(NQ):
                for hp in range(MQ):
                    blk = q * MQ + hp
                    for m in range(SC):
                        gtime = SC * blk + m
                        ss = psS.tile([P, 2, NB], F32, tag="s")
                        for j in range(2):
                            lo, hi = j * HEAD_DIM, (j + 1) * HEAD_DIM
                            nc.tensor.matmul(
                                ss[:, j, :],
                                lhsT=kt_sb[lo:hi, hp, m * P:(m + 1) * P],
                                rhs=qt_sb[lo:hi, hp, q * NB:(q + 1) * NB],
                                start=True, stop=True,
                            )
                        nc.scalar.activation(es_sb[:, m % ESR, :, :], ss[:],
                                             Exp, scale=SCALE)
                        if blk == 0 and 1 <= m <= 3:
                            qk_proj_block(xk_sb, wk_sb, kt_sb, 0, m, False)
                        if blk == 0 and 4 <= m <= 7:
                            qk_proj_block(xk_sb, wk_sb, kt_sb, 1, m - 4, False)
                        if blk == 0 and m == 8:
                            qk_proj_block(xq_sb, wq_sb, qt_sb, 1, 0, True)
                        if hp == 1 and q + 1 < NQ and 5 <= m <= 7:
                            qk_proj_block(xq_sb, wq_sb, qt_sb, 0, q + 1, True,
                                          ks=range(2 * (m - 5), 2 * (m - 4)))
                        if hp == 1 and q + 1 < NQ and 11 <= m <= 13:
                            qk_proj_block(xq_sb, wq_sb, qt_sb, 1, q + 1, True,
                                          ks=range(2 * (m - 11), 2 * (m - 10)))
                        drain(gtime, 2)
                        if m % 2 == 1 and m >= 5 and state["oq"]:
                            qd = state["oq"][0]
                            if out_proj_half.done[qd] < 8:
                                nn = out_proj_half.done[qd]
                                out_proj_half(qd * 4 + nn // 2, nn % 2)
                                out_proj_half.done[qd] = nn + 1
                            else:
                                state["oq"].pop(0)

            # --- tail: remaining PV chunks, normalizes, out-projections ---
            gtime = NCORES * SC
            while state["pv"] < NCORES * SC:
                drain(gtime, 2)
                gtime += 1
            for q in range(NQ):
                while out_proj_half.done[q] < 8:
                    nn = out_proj_half.done[q]
                    out_proj_half(q * 4 + nn // 2, nn % 2)
                    out_proj_half.done[q] = nn + 1

    if split_waits:
        _split_multi_waits(nc)
    return nc


_NC = None


def _get_nc():
    global _NC
    if _NC is None:
        _NC = build_nc()
    return _NC


def _bf(a):
    return np.ascontiguousarray(np.asarray(a, dtype=np.float32)).astype(NPBF16)


def make_in_maps(query, key, value, wq, bq, wk, bk, wv, bv, wo, bo):
    query = np.asarray(query, np.float32)
    key = np.asarray(key, np.float32)
    value = np.asarray(value, np.float32)
    wqf = np.asarray(wq, np.float32)
    wkf = np.asarray(wk, np.float32)
    wvf = np.asarray(wv, np.float32)
    wof = np.asarray(wo, np.float32)
    in_maps = []
    for b in range(B):
        xqT = _bf(query[b].T)
        xkT = _bf(key[b].T)
        xvT = _bf(value[b].T)
        for g in range(GROUPS):
            sl = slice(g * GE, (g + 1) * GE)
            in_maps.append({
                "xqT": xqT,
                "xkT": xkT,
                "xvT": xvT,
                "wq": _bf(wqf[:, sl]),
                "wk": _bf(wkf[:, sl]),
                "wv": _bf(wvf[:, sl]),
                "wo": _bf(wof[sl, :]),
                "bq": np.ascontiguousarray(np.asarray(bq, np.float32)[sl]),
            })
    return in_maps


def kernel(query, key, value, wq, bq, wk, bk, wv, bv, wo, bo, **kw):
    nc = _get_nc()
    in_maps = make_in_maps(query, key, value, wq, bq, wk, bk, wv, bv, wo, bo)
    res = run_bass_kernel_spmd(nc, in_maps, list(range(NCORES))).results
    # bk is softmax-invariant; bv rides through softmax into a constant
    bo_eff = (np.asarray(bo, np.float32)
              + np.asarray(bv, np.float32) @ np.asarray(wo, np.float32))
    outp = np.empty((B, S, DIM), np.float32)
    for b in range(B):
        outp[b] = (res[b * GROUPS]["out"].astype(np.float32)
                   + res[b * GROUPS + 1]["out"].astype(np.float32) + bo_eff)
    return outp
